# revision 18
# baseline (speedup 1.0000x reference)
"""Trainium2 Bass kernel for a dense transformer decoder layer.

Tensor-parallel across 8 NeuronCores:
  - heads: 2 per core (of 16), ff channels: 1024 per core (of 8192)
  - W_in rows / W_out cols sharded accordingly; per-(chunk, oc) bf16
    ReduceScatter(add) of the partial outputs; host concatenates.

Per-core dataflow (token chunks, 512 except two 256 tail chunks so the
final reduce-scatter is small):
  the RMSNorm scale s = rsqrt(mean(x^2)+eps) is computed on the HOST in
  fp32 (exactly like the reference) and shipped pre-broadcast: sbc
  [128, T] for the matmul-eviction scaling, scols [128, T/128] for the
  token-major v eviction.  norm_w is folded into W on the host; the
  normed_ages overwrite is pre-patched into the last two hid rows of
  the transposed x (a12 / s, so the eviction scale restores a12).
  W_in matmul produces q/k transposed ([hd, tok]; rope via a pairwise
  swap matmul + two multiplies), v token-major via a second matmul
  orientation, and the swiglu branch.  Causal attention with
  k-token-major score tiles, a one-deep score lookahead so the PE never
  waits on the exp, exp without max-subtraction, an aligned [128,128]
  triangle mask on the diagonal block only, and fully-masked P*V blocks
  skipped.  Softmax denominator rides along as an appended ones-column
  on v.  The combined [ff|attn] activations feed the W_out matmul,
  evicted in bf16 and reduce-scattered per 512-wide output column group
  for fine-grained comm overlap.
"""

import os
import sys

for _p in ("/opt/trn_rl_repo", "/opt/pypackages"):
    if _p not in sys.path:
        sys.path.insert(0, _p)

import numpy as np
import ml_dtypes

BF16 = ml_dtypes.bfloat16

# Model dims (fixed by the problem)
T_FULL = 4096
HID = 2048
NH = 16
HD = 128
INTER = 8192
EPS = 1e-6
SCALE = 1.0 / float(np.sqrt(np.float32(HD)))

NCORES = 8
HPC = NH // NCORES          # heads per core = 2
FPC = INTER // NCORES       # ff channels per core = 1024
NFF = FPC // 128            # ff m-tiles per core (per g1/g2) = 8
NCOMB = NFF + HPC           # comb k-tiles: ff + one per head = 10
KH = HID // 128             # hid k-tiles = 16

# physical 512-token chunks (vproj / W_in / rope run at full matmul
# efficiency); the LAST physical chunk's attention/W_out/reduce-scatter
# is split into sub-chunks so the final collective (the kernel's tail)
# is short
PHYS_TC = 512
NPHYS = T_FULL // PHYS_TC
SUBS = [[(c * PHYS_TC, PHYS_TC)] for c in range(NPHYS)]
SUBS[-1] = [(T_FULL - 512, 256), (T_FULL - 256, 128),
            (T_FULL - 128, 128)]


def _build_nc(T):
    import concourse.bass as bass
    import concourse.tile as tile
    from concourse import bacc, mybir

    f32 = mybir.dt.float32
    bf16 = mybir.dt.bfloat16
    AF = mybir.ActivationFunctionType

    NO = HID // 512              # output col chunks = 4
    JT = T // 128                # total k-blocks (tok tiles) over full T
    chunk_list = [(c * PHYS_TC, PHYS_TC) for c in range(NPHYS)]

    nc = bacc.Bacc("TRN2", target_bir_lowering=False, debug=False,
                   num_devices=NCORES)

    # ---- DRAM parameters -------------------------------------------------
    xT_d = nc.dram_tensor("xt", [HID, T], bf16, kind="ExternalInput").ap()
    sbc_d = nc.dram_tensor("sbc", [128, T], f32, kind="ExternalInput").ap()
    scols_d = nc.dram_tensor("scols", [128, JT], f32,
                             kind="ExternalInput").ap()
    win_d = nc.dram_tensor("w_in_t", [2 * NFF + 2 * HPC, 128, KH, 128], bf16,
                           kind="ExternalInput").ap()
    wv_d = nc.dram_tensor("w_v_t", [128, KH, HPC * 128], bf16,
                          kind="ExternalInput").ap()
    wo_d = nc.dram_tensor("w_out_t", [NO, 128, NCOMB, 512], bf16,
                          kind="ExternalInput").ap()
    cos_d = nc.dram_tensor("cos_t", [HD, T], bf16, kind="ExternalInput").ap()
    sin_d = nc.dram_tensor("sin_t", [HD, T], bf16, kind="ExternalInput").ap()
    swap_d = nc.dram_tensor("swapmat", [128, 128], bf16,
                            kind="ExternalInput").ap()
    mask_d = nc.dram_tensor("maskbase", [128, 896], bf16,
                            kind="ExternalInput").ap()
    ident_d = nc.dram_tensor("identity", [128, 128], bf16,
                             kind="ExternalInput").ap()
    # flat output: rows indexed by tok0//8 + t within each chunk segment
    out_d = nc.dram_tensor("out", [NO, T // NCORES, 512], bf16,
                           kind="ExternalOutput").ap()

    from contextlib import ExitStack

    with tile.TileContext(nc) as tc_ctx:
        with ExitStack() as ctx:
            const = ctx.enter_context(tc_ctx.tile_pool(name="const", bufs=1))
            kv = ctx.enter_context(tc_ctx.tile_pool(name="kv", bufs=1))
            dram = ctx.enter_context(
                tc_ctx.tile_pool(name="dram", bufs=1, space="DRAM"))
            xpool = ctx.enter_context(tc_ctx.tile_pool(name="xpool", bufs=2))
            spool = ctx.enter_context(tc_ctx.tile_pool(name="spool", bufs=2))
            wmp = ctx.enter_context(tc_ctx.tile_pool(name="wmp", bufs=6))
            evictp = ctx.enter_context(
                tc_ctx.tile_pool(name="evictp", bufs=6))
            qkp = ctx.enter_context(tc_ctx.tile_pool(name="qkp", bufs=4))
            combp = ctx.enter_context(
                tc_ctx.tile_pool(name="combp", bufs=NCOMB + 2))
            ppool = ctx.enter_context(tc_ctx.tile_pool(name="ppool", bufs=4))
            attnp = ctx.enter_context(tc_ctx.tile_pool(name="attnp", bufs=4))
            wop = ctx.enter_context(tc_ctx.tile_pool(name="wop", bufs=4))
            outp = ctx.enter_context(tc_ctx.tile_pool(name="outp", bufs=4))
            ps_mm = ctx.enter_context(
                tc_ctx.tile_pool(name="ps_mm", bufs=2, space="PSUM"))
            ps_misc = ps_mm
            ps_attn = ctx.enter_context(
                tc_ctx.tile_pool(name="ps_attn", bufs=4, space="PSUM"))
            ps_out = ctx.enter_context(
                tc_ctx.tile_pool(name="ps_out", bufs=2, space="PSUM"))
            # ---- constants ----------------------------------------------
            swap_sb = const.tile([128, 128], bf16, name="swap_sb")
            nc.scalar.dma_start(out=swap_sb, in_=swap_d)
            mask_sb = const.tile([128, 896], bf16, name="mask_sb")
            nc.scalar.dma_start(out=mask_sb, in_=mask_d)
            tri_sb = mask_sb[:, 384:512]
            ident_sb = const.tile([128, 128], bf16, name="ident_sb")
            nc.scalar.dma_start(out=ident_sb, in_=ident_d)
            dummy_sb = const.tile([1, 1], f32, name="dummy_sb")
            nc.vector.memset(dummy_sb, 0.0)
            # per-token rms scale, token-major columns (for v eviction)
            scols_sb = const.tile([128, JT], f32, name="scols_sb")
            nc.scalar.dma_start(out=scols_sb, in_=scols_d)
            # v-projection weights, resident: [128 hid-part, KH, HPC*128]
            wv_sb = const.tile([128, KH, HPC * 128], bf16, name="wv_sb")
            nc.sync.dma_start(out=wv_sb, in_=wv_d)

            # persistent K / V (token history)
            kT = kv.tile([128, HPC, T], bf16, name="kT")
            v_sb = kv.tile([128, HPC, JT, 129], bf16, name="v_sb")

            # ---- per-chunk helper emitters ------------------------------
            def emit_sbc(ci):
                tok0, tc = chunk_list[ci]
                s_bc = spool.tile([128, 512], f32, tag="sbc", bufs=2,
                                  name=f"sbc_{ci}")
                nc.scalar.dma_start(out=s_bc[:, 0:tc],
                                    in_=sbc_d[:, tok0:tok0 + tc])
                return s_bc

            def emit_xt_loads(ci):
                """hid-major x tiles, one fused DMA (sync queue); ages
                rows pre-patched by the host."""
                tok0, tc = chunk_list[ci]
                xall = xpool.tile([128, KH, 512], bf16, tag="xT",
                                  name=f"xT_{ci}")
                for k in range(KH):
                    nc.sync.dma_start(
                        out=xall[:, k, 0:tc],
                        in_=xT_d[k * 128:(k + 1) * 128, tok0:tok0 + tc])
                return xall

            def emit_cos_sin(ci):
                tok0, tc = chunk_list[ci]
                cos_sb = qkp.tile([128, 512], bf16, tag="cos", bufs=2,
                                  name=f"cos_{ci}")
                nc.scalar.dma_start(out=cos_sb[:, 0:tc],
                                    in_=cos_d[:, tok0:tok0 + tc])
                sin_sb = qkp.tile([128, 512], bf16, tag="sin", bufs=2,
                                  name=f"sin_{ci}")
                nc.scalar.dma_start(out=sin_sb[:, 0:tc],
                                    in_=sin_d[:, tok0:tok0 + tc])
                return cos_sb, sin_sb

            # ---- chunk 0 prologue ---------------------------------------
            s_bc = emit_sbc(0)
            xTt = emit_xt_loads(0)
            cos_sb, sin_sb = emit_cos_sin(0)

            rs_tiles = []
            for ci, (tok0, tc) in enumerate(chunk_list):
                last = ci == len(chunk_list) - 1
                NT = tc // 128

                # ---- W_out weight prefetch ------------------------------
                wots = []
                for oc in range(NO):
                    wot = wop.tile([128, NCOMB, 512], bf16, tag="wo",
                                   name=f"wo_{ci}_{oc}")
                    nc.scalar.dma_start(out=wot, in_=wo_d[oc])
                    wots.append(wot)

                # ---- v projection (token-major) -------------------------
                for tsub in range(NT):
                    pv = ps_mm.tile([128, HPC * 128], f32, tag="a",
                                    name=f"pv_{ci}_{tsub}")
                    for k in range(KH):
                        nc.tensor.matmul(
                            pv,
                            lhsT=xTt[:, k, tsub * 128:(tsub + 1) * 128],
                            rhs=wv_sb[:, k, :],
                            start=(k == 0), stop=(k == KH - 1))
                    j = tok0 // 128 + tsub
                    for h in range(HPC):
                        nc.vector.tensor_scalar_mul(
                            v_sb[:, h, j, 0:128], pv[:, h * 128:(h + 1) * 128],
                            scols_sb[:, j:j + 1])
                        nc.vector.memset(v_sb[:, h, j, 128:129], 1.0)

                # ---- fused W_in matmul (transposed out) -----------------
                # m order: g1_0, g2_0, ..., g1_7, g2_7, qA, qB, kA, kB
                silu_prev = None
                qk_raw = {}
                comb = [None] * NCOMB
                for m in range(2 * NFF + 2 * HPC):
                    wmt = wmp.tile([128, KH, 128], bf16, tag="wm",
                                   name=f"wm_{ci}_{m}")
                    nc.sync.dma_start(out=wmt, in_=win_d[m])
                    pm = ps_mm.tile([128, tc], f32, tag="a",
                                    name=f"pm_{ci}_{m}")
                    for k in range(KH):
                        nc.tensor.matmul(pm, lhsT=wmt[:, k, :],
                                         rhs=xTt[:, k, 0:tc],
                                         start=(k == 0),
                                         stop=(k == KH - 1))
                    if m < 2 * NFF and m % 2 == 0:      # g1
                        g1t = evictp.tile([128, 512], bf16, tag="g1",
                                          name=f"g1_{ci}_{m//2}")
                        nc.vector.tensor_mul(g1t[:, 0:tc], pm, s_bc[:, 0:tc])
                        st = evictp.tile([128, 512], bf16, tag="silu",
                                         name=f"silu_{ci}_{m//2}")
                        nc.scalar.activation(st[:, 0:tc], g1t[:, 0:tc],
                                             AF.Silu)
                        silu_prev = st
                    elif m < 2 * NFF:                    # g2
                        p = m // 2
                        g2t = evictp.tile([128, 512], bf16, tag="g2",
                                          name=f"g2_{ci}_{p}")
                        nc.vector.tensor_mul(g2t[:, 0:tc], pm, s_bc[:, 0:tc])
                        ct = combp.tile([128, 512], bf16, tag="comb",
                                        name=f"comb_{ci}_{p}")
                        nc.vector.tensor_mul(ct[:, 0:tc], silu_prev[:, 0:tc],
                                             g2t[:, 0:tc])
                        comb[p] = ct
                    else:                                # q or k
                        qi = m - 2 * NFF
                        qk_raw[qi] = qkp.tile([128, 512], bf16, tag="qkraw",
                                              name=f"qkraw_{ci}_{qi}")
                        nc.vector.tensor_mul(qk_raw[qi][:, 0:tc], pm,
                                             s_bc[:, 0:tc])

                # warm the Exp table off the critical path
                dwarm = spool.tile([1, 1], f32, tag="dwarm", bufs=2,
                                   name=f"dwarm_{ci}")
                nc.scalar.activation(dwarm, dummy_sb, AF.Exp, scale=SCALE)

                # ---- rope ----------------------------------------------
                qT = qkp.tile([128, HPC, 512], bf16, tag="qT", bufs=2,
                              name=f"qT_{ci}")
                # (qi, destination slice): q -> qT chunk, k -> resident kT
                rope_jobs = [(h, qT[:, h, 0:tc]) for h in range(HPC)]
                rope_jobs += [(HPC + h, kT[:, h, tok0:tok0 + tc])
                              for h in range(HPC)]
                for qi, dst in rope_jobs:
                    src = qk_raw[qi]
                    psw = ps_misc.tile([128, tc], f32, tag="a",
                                       name=f"psw_{ci}_{qi}")
                    nc.tensor.matmul(psw, lhsT=swap_sb, rhs=src[:, 0:tc],
                                     start=True, stop=True)
                    rt1 = qkp.tile([128, 512], bf16, tag="rt1", bufs=2,
                                   name=f"rt1_{ci}_{qi}")
                    nc.vector.tensor_mul(rt1[:, 0:tc], psw, sin_sb[:, 0:tc])
                    rt2 = qkp.tile([128, 512], bf16, tag="rt2", bufs=2,
                                   name=f"rt2_{ci}_{qi}")
                    nc.vector.tensor_mul(rt2[:, 0:tc], src[:, 0:tc],
                                         cos_sb[:, 0:tc])
                    nc.vector.tensor_add(dst, rt1[:, 0:tc], rt2[:, 0:tc])
                qk_raw = {}

                # ---- prefetch next chunk inputs (early issue slots) -----
                if not last:
                    s_bc_n = emit_sbc(ci + 1)
                    xTt_n = emit_xt_loads(ci + 1)
                    cs_n = emit_cos_sin(ci + 1)

                # ---- causal attention + output proj, per sub-chunk ------
                for (q0, tcs) in SUBS[ci]:
                    qrel = q0 - tok0
                    j0 = q0 // 128        # first diagonal k-block index
                    NB = tcs // 128
                    kmax = (q0 + tcs) // 128
                    seg = tcs // NCORES
                    for h in range(HPC):
                        pa = [ps_attn.tile([128, 129], f32, tag="attn",
                                           name=f"pa_{ci}_{q0}_{h}_{i}")
                              for i in range(NB)]

                        def emit_score(j):
                            psc = ps_misc.tile([128, tcs], f32, tag="a",
                                               name=f"psc_{ci}_{q0}_{h}_{j}")
                            nc.tensor.matmul(
                                psc, lhsT=kT[:, h, j * 128:(j + 1) * 128],
                                rhs=qT[:, h, qrel:qrel + tcs],
                                start=True, stop=True)
                            return psc

                        pscs = [emit_score(0)]
                        for j in range(kmax):
                            psc_cur = pscs[j]
                            if j + 1 < kmax:
                                pscs.append(emit_score(j + 1))
                            pT = ppool.tile([128, 512], bf16, tag="p",
                                            name=f"pT_{ci}_{q0}_{h}_{j}")
                            nc.scalar.activation(pT[:, 0:tcs], psc_cur,
                                                 AF.Exp, scale=SCALE)
                            D = j * 128 - q0
                            if D >= 0:
                                # triangle mask on the diagonal block only
                                nc.vector.tensor_mul(
                                    pT[:, D:D + 128], pT[:, D:D + 128],
                                    tri_sb)
                            for b in range(NB):
                                jmax_b = j0 + b
                                if j > jmax_b:
                                    continue  # fully-masked block: skip
                                nc.tensor.matmul(
                                    pa[b],
                                    lhsT=pT[:, b * 128:(b + 1) * 128],
                                    rhs=v_sb[:, h, j, :],
                                    start=(j == 0), stop=(j == jmax_b))
                        # normalize + transpose into comb tiles
                        for b in range(NB):
                            li = attnp.tile([128, 1], f32, tag="l",
                                            name=f"l_{ci}_{q0}_{h}_{b}")
                            nc.vector.reciprocal(li, pa[b][:, 128:129])
                            at = attnp.tile([128, 128], bf16, tag="at",
                                            name=f"at_{ci}_{q0}_{h}_{b}")
                            nc.vector.tensor_scalar_mul(
                                at, pa[b][:, 0:128], li)
                            ptr = ps_misc.tile([128, 128], bf16, tag="a",
                                               name=f"ptr_{ci}_{q0}_{h}_{b}")
                            nc.tensor.transpose(ptr, at, ident_sb)
                            if comb[NFF + h] is None:
                                comb[NFF + h] = combp.tile(
                                    [128, 512], bf16, tag="comb",
                                    name=f"comb_at_{ci}_{h}")
                            col0 = qrel + b * 128
                            nc.vector.tensor_copy(
                                comb[NFF + h][:, col0:col0 + 128], ptr)

                    # ---- output projection for this sub-chunk -----------
                    NTS = tcs // 128
                    whole = tcs == 512
                    if whole:
                        acc_oc = [dram.tile([tcs, 512], bf16, tag="acc",
                                            bufs=8,
                                            name=f"acc_{ci}_{q0}_{oc}")
                                  for oc in range(NO)]
                    else:
                        acc_one = dram.tile([tcs, HID], bf16,
                                            tag=f"acc1_{tcs}", bufs=2,
                                            name=f"acc1_{ci}_{q0}")
                    for oc in range(NO):
                        wot = wots[oc]
                        for tsub in range(NTS):
                            po = ps_out.tile([128, 512], f32, tag="out",
                                             name=f"po_{ci}_{q0}_{oc}_{tsub}")
                            csl = slice(qrel + tsub * 128,
                                        qrel + (tsub + 1) * 128)
                            for kc in range(NCOMB):
                                nc.tensor.matmul(
                                    po, lhsT=comb[kc][:, csl],
                                    rhs=wot[:, kc, :],
                                    start=(kc == 0), stop=(kc == NCOMB - 1))
                            ost = outp.tile([128, 512], bf16, tag="ost",
                                            name=f"ost_{ci}_{q0}_{oc}_{tsub}")
                            nc.vector.tensor_copy(ost, po)
                            r0 = tsub * 128
                            if whole:
                                nc.gpsimd.dma_start(
                                    out=acc_oc[oc][r0:r0 + 128, :], in_=ost)
                            else:
                                nc.gpsimd.dma_start(
                                    out=acc_one[r0:r0 + 128,
                                                oc * 512:(oc + 1) * 512],
                                    in_=ost)

                    # ---- reduce-scatter this sub-chunk ------------------
                    if whole:
                        for oc in range(NO):
                            rs_c = dram.tile([seg, 512], bf16, tag="rs",
                                             bufs=32,
                                             name=f"rs_{ci}_{q0}_{oc}")
                            nc.gpsimd.collective_compute(
                                "ReduceScatter",
                                mybir.AluOpType.add,
                                replica_groups=[list(range(NCORES))],
                                ins=[acc_oc[oc][:, :]],
                                outs=[rs_c[:, :]],
                            )
                            rs_tiles.append((q0, tcs, oc, rs_c))
                    else:
                        rs_c = dram.tile([seg, HID], bf16,
                                         tag=f"rs1_{tcs}", bufs=2,
                                         name=f"rs1_{ci}_{q0}")
                        nc.gpsimd.collective_compute(
                            "ReduceScatter",
                            mybir.AluOpType.add,
                            replica_groups=[list(range(NCORES))],
                            ins=[acc_one[:, :]],
                            outs=[rs_c[:, :]],
                        )
                        rs_tiles.append((q0, tcs, None, rs_c))

                if not last:
                    s_bc = s_bc_n
                    xTt = xTt_n
                    cos_sb, sin_sb = cs_n

            # deferred final output DMAs; gpsimd SWDGE path so the HW DGE
            # queue counters never chain later DMAs behind the collectives
            for (q0, tcs, oc, rs_c) in rs_tiles:
                r0 = q0 // NCORES
                if oc is not None:
                    nc.gpsimd.dma_start(
                        out=out_d[oc, r0:r0 + tcs // NCORES, :],
                        in_=rs_c[:, :])
                else:
                    for o2 in range(NO):
                        nc.gpsimd.dma_start(
                            out=out_d[o2, r0:r0 + tcs // NCORES, :],
                            in_=rs_c[:, o2 * 512:(o2 + 1) * 512])

    nc.compile()
    return nc


def _prep_in_maps(x, normed_ages, sin, cos, norm_w, W_in, W_out):
    """Shard + preprocess inputs into per-core in_maps (numpy only)."""
    T = x.shape[0]
    x = np.asarray(x, np.float32)
    # host-side RMSNorm scale, fp32 exactly like the reference
    s = 1.0 / np.sqrt(np.mean(x * x, axis=1) + EPS)          # [T]
    sbc = np.ascontiguousarray(
        np.broadcast_to(s[None, :], (128, T))).astype(np.float32)
    scols = np.ascontiguousarray(
        s.reshape(T // 128, 128).T).astype(np.float32)       # [128, JT]

    xT_bf = np.ascontiguousarray(x.T).astype(BF16)
    # ages overwrite: patch the last two hid rows with a12 / s so the
    # eviction-side multiply by s restores a12 exactly
    a1 = np.asarray(normed_ages, np.float32)
    xT_bf[HID - 2, :] = (a1 / s).astype(BF16)
    xT_bf[HID - 1, :] = (a1 * a1 / s).astype(BF16)

    cos_t = np.ascontiguousarray(cos.reshape(T, HD).T).astype(BF16)
    sin_t = np.ascontiguousarray(sin.reshape(T, HD).T).astype(BF16)

    sw = np.zeros((128, 128), np.float32)
    idx = np.arange(0, 128, 2)
    sw[idx + 1, idx] = -1.0   # lhsT[2i+1, 2i] = -1
    sw[idx, idx + 1] = 1.0    # lhsT[2i, 2i+1] = +1
    swapmat = sw.astype(BF16)

    maskbase = (np.arange(896)[None, :] - 384 >=
                np.arange(128)[:, None]).astype(BF16)
    identity = np.eye(128, dtype=np.float32).astype(BF16)

    # norm_w folded into W_in except the last two hid columns (the
    # normed_ages overwrite bypasses the norm weight).
    def fold(wrows):
        w = wrows * norm_w[None, :]
        w[:, HID - 2:] = wrows[:, HID - 2:]
        return w

    q_base = 2 * INTER
    k_base = 2 * INTER + HID
    v_base = 2 * INTER + 2 * HID

    in_maps = []
    for core in range(NCORES):
        f0 = FPC * core
        h0 = HPC * core
        rows = []
        for p in range(NFF):
            rows.append(W_in[f0 + p * 128: f0 + (p + 1) * 128])           # g1_p
            rows.append(W_in[INTER + f0 + p * 128:
                             INTER + f0 + (p + 1) * 128])                 # g2_p
        for h in range(HPC):
            rows.append(W_in[q_base + (h0 + h) * HD:
                             q_base + (h0 + h + 1) * HD])                 # q
        for h in range(HPC):
            rows.append(W_in[k_base + (h0 + h) * HD:
                             k_base + (h0 + h + 1) * HD])                 # k
        w_used = fold(np.concatenate(rows, axis=0))                       # [2560, HID]
        nm = 2 * NFF + 2 * HPC
        # [m, p(hid-in-tile), k, j(row-in-tile)] so each partition is linear
        w_in_t = np.ascontiguousarray(
            w_used.reshape(nm, 128, KH, 128).transpose(0, 3, 2, 1)
        ).astype(BF16)

        wv = fold(W_in[v_base + h0 * HD: v_base + (h0 + HPC) * HD])       # [256, HID]
        w_v_t = np.ascontiguousarray(
            wv.reshape(HPC * 128, KH, 128).transpose(2, 1, 0)).astype(BF16)

        # W_out columns in comb order: ff block, then attn heads
        cols = list(range(HID + f0, HID + f0 + FPC))
        for h in range(HPC):
            cols += list(range((h0 + h) * HD, (h0 + h + 1) * HD))
        w_o_loc_t = np.ascontiguousarray(W_out[:, cols].T)                # [1280, HID]
        # [oc, p(c-in-tile), kc, ow] so each partition is linear per oc
        w_out_t = np.ascontiguousarray(
            w_o_loc_t.reshape(NCOMB, 128, HID // 512, 512)
            .transpose(2, 1, 0, 3)).astype(BF16)

        in_maps.append({
            "xt": xT_bf, "sbc": sbc, "scols": scols,
            "w_in_t": w_in_t, "w_v_t": w_v_t, "w_out_t": w_out_t,
            "cos_t": cos_t, "sin_t": sin_t,
            "swapmat": swapmat, "maskbase": maskbase, "identity": identity,
        })
    return in_maps


_NC_CACHE = {}


def get_nc(T=T_FULL):
    if T not in _NC_CACHE:
        _NC_CACHE[T] = _build_nc(T)
    return _NC_CACHE[T]


def run(x, normed_ages, sin, cos, norm_w, W_in, W_out, T=T_FULL,
        trace=False):
    from concourse.bass_utils import run_bass_kernel_spmd
    nc = get_nc(T)
    in_maps = _prep_in_maps(x, normed_ages, sin, cos, norm_w, W_in, W_out)
    res = run_bass_kernel_spmd(nc, in_maps, list(range(NCORES)), trace=trace)
    # results[i]["out"][oc, tok0//8 + t] holds reduced rows
    # [tok0 + i*seg + t, oc*512:(oc+1)*512] for each chunk
    out = np.empty((T, HID), np.float32)
    for i in range(NCORES):
        oi = np.asarray(res.results[i]["out"], np.float32)
        for subs in SUBS:
            for (q0, tcs) in subs:
                seg = tcs // NCORES
                r0 = q0 // NCORES
                for oc in range(HID // 512):
                    out[q0 + i * seg: q0 + (i + 1) * seg,
                        oc * 512:(oc + 1) * 512] = oi[oc, r0:r0 + seg]
    return out, res


def kernel(x, normed_ages, sin, cos, norm_w, W_in, W_out):
    out, _ = run(x, normed_ages, sin, cos, norm_w, W_in, W_out)
    return out


if __name__ == "__main__":
    import reference
    inputs = reference.setup_inputs()
    inputs = {k: np.asarray(v) for k, v in inputs.items()}
    expected = np.asarray(reference.reference(**inputs))
    got = kernel(**inputs)
    rel = np.linalg.norm(got - expected) / np.linalg.norm(expected)
    print("rel", rel)


# revision 19
# speedup vs baseline: 1.0053x; 1.0053x over previous
"""Trainium2 Bass kernel for a dense transformer decoder layer.

Tensor-parallel across 8 NeuronCores:
  - heads: 2 per core (of 16), ff channels: 1024 per core (of 8192)
  - W_in rows / W_out cols sharded accordingly; per-(chunk, oc) bf16
    ReduceScatter(add) of the partial outputs; host concatenates.

Per-core dataflow (token chunks, 512 except two 256 tail chunks so the
final reduce-scatter is small):
  the RMSNorm scale s = rsqrt(mean(x^2)+eps) is computed on the HOST in
  fp32 (exactly like the reference) and shipped pre-broadcast: sbc
  [128, T] for the matmul-eviction scaling, scols [128, T/128] for the
  token-major v eviction.  norm_w is folded into W on the host; the
  normed_ages overwrite is pre-patched into the last two hid rows of
  the transposed x (a12 / s, so the eviction scale restores a12).
  W_in matmul produces q/k transposed ([hd, tok]; rope via a pairwise
  swap matmul + two multiplies), v token-major via a second matmul
  orientation, and the swiglu branch.  Causal attention with
  k-token-major score tiles, a one-deep score lookahead so the PE never
  waits on the exp, exp without max-subtraction, an aligned [128,128]
  triangle mask on the diagonal block only, and fully-masked P*V blocks
  skipped.  Softmax denominator rides along as an appended ones-column
  on v.  The combined [ff|attn] activations feed the W_out matmul,
  evicted in bf16 and reduce-scattered per 512-wide output column group
  for fine-grained comm overlap.
"""

import os
import sys

for _p in ("/opt/trn_rl_repo", "/opt/pypackages"):
    if _p not in sys.path:
        sys.path.insert(0, _p)

import numpy as np
import ml_dtypes

BF16 = ml_dtypes.bfloat16

# Model dims (fixed by the problem)
T_FULL = 4096
HID = 2048
NH = 16
HD = 128
INTER = 8192
EPS = 1e-6
SCALE = 1.0 / float(np.sqrt(np.float32(HD)))

NCORES = 8
HPC = NH // NCORES          # heads per core = 2
FPC = INTER // NCORES       # ff channels per core = 1024
NFF = FPC // 128            # ff m-tiles per core (per g1/g2) = 8
NCOMB = NFF + HPC           # comb k-tiles: ff + one per head = 10
KH = HID // 128             # hid k-tiles = 16

# physical 512-token chunks (vproj / W_in / rope run at full matmul
# efficiency); the LAST physical chunk's attention/W_out/reduce-scatter
# is split into sub-chunks so the final collective (the kernel's tail)
# is short
PHYS_TC = 512
NPHYS = T_FULL // PHYS_TC
SUBS = [[(c * PHYS_TC, PHYS_TC)] for c in range(NPHYS)]
SUBS[-1] = [(T_FULL - 512, 256), (T_FULL - 256, 128),
            (T_FULL - 128, 128)]


def _build_nc(T):
    import concourse.bass as bass
    import concourse.tile as tile
    from concourse import bacc, mybir

    f32 = mybir.dt.float32
    bf16 = mybir.dt.bfloat16
    AF = mybir.ActivationFunctionType

    NO = HID // 512              # output col chunks = 4
    JT = T // 128                # total k-blocks (tok tiles) over full T
    chunk_list = [(c * PHYS_TC, PHYS_TC) for c in range(NPHYS)]

    nc = bacc.Bacc("TRN2", target_bir_lowering=False, debug=False,
                   num_devices=NCORES)

    # ---- DRAM parameters -------------------------------------------------
    xT_d = nc.dram_tensor("xt", [HID, T], bf16, kind="ExternalInput").ap()
    sbc_d = nc.dram_tensor("sbc", [128, T], f32, kind="ExternalInput").ap()
    scols_d = nc.dram_tensor("scols", [128, JT], f32,
                             kind="ExternalInput").ap()
    win_d = nc.dram_tensor("w_in_t", [2 * NFF + 2 * HPC, 128, KH, 128], bf16,
                           kind="ExternalInput").ap()
    wv_d = nc.dram_tensor("w_v_t", [128, KH, HPC * 128], bf16,
                          kind="ExternalInput").ap()
    wo_d = nc.dram_tensor("w_out_t", [NO, 128, NCOMB, 512], bf16,
                          kind="ExternalInput").ap()
    cos_d = nc.dram_tensor("cos_t", [HD, T], bf16, kind="ExternalInput").ap()
    sin_d = nc.dram_tensor("sin_t", [HD, T], bf16, kind="ExternalInput").ap()
    swap_d = nc.dram_tensor("swapmat", [128, 128], bf16,
                            kind="ExternalInput").ap()
    mask_d = nc.dram_tensor("maskbase", [128, 896], bf16,
                            kind="ExternalInput").ap()
    ident_d = nc.dram_tensor("identity", [128, 128], bf16,
                             kind="ExternalInput").ap()
    # flat output: rows indexed by tok0//8 + t within each chunk segment
    out_d = nc.dram_tensor("out", [NO, T // NCORES, 512], bf16,
                           kind="ExternalOutput").ap()

    from contextlib import ExitStack

    with tile.TileContext(nc) as tc_ctx:
        with ExitStack() as ctx:
            const = ctx.enter_context(tc_ctx.tile_pool(name="const", bufs=1))
            kv = ctx.enter_context(tc_ctx.tile_pool(name="kv", bufs=1))
            dram = ctx.enter_context(
                tc_ctx.tile_pool(name="dram", bufs=1, space="DRAM"))
            xpool = ctx.enter_context(tc_ctx.tile_pool(name="xpool", bufs=2))
            spool = ctx.enter_context(tc_ctx.tile_pool(name="spool", bufs=2))
            wmp = ctx.enter_context(tc_ctx.tile_pool(name="wmp", bufs=6))
            evictp = ctx.enter_context(
                tc_ctx.tile_pool(name="evictp", bufs=6))
            qkp = ctx.enter_context(tc_ctx.tile_pool(name="qkp", bufs=4))
            combp = ctx.enter_context(
                tc_ctx.tile_pool(name="combp", bufs=NCOMB + 2))
            ppool = ctx.enter_context(tc_ctx.tile_pool(name="ppool", bufs=4))
            attnp = ctx.enter_context(tc_ctx.tile_pool(name="attnp", bufs=4))
            wop = ctx.enter_context(tc_ctx.tile_pool(name="wop", bufs=4))
            outp = ctx.enter_context(tc_ctx.tile_pool(name="outp", bufs=4))
            ps_mm = ctx.enter_context(
                tc_ctx.tile_pool(name="ps_mm", bufs=2, space="PSUM"))
            ps_misc = ps_mm
            ps_attn = ctx.enter_context(
                tc_ctx.tile_pool(name="ps_attn", bufs=4, space="PSUM"))
            ps_out = ctx.enter_context(
                tc_ctx.tile_pool(name="ps_out", bufs=2, space="PSUM"))
            # ---- constants ----------------------------------------------
            swap_sb = const.tile([128, 128], bf16, name="swap_sb")
            nc.scalar.dma_start(out=swap_sb, in_=swap_d)
            mask_sb = const.tile([128, 896], bf16, name="mask_sb")
            nc.scalar.dma_start(out=mask_sb, in_=mask_d)
            tri_sb = mask_sb[:, 384:512]
            ident_sb = const.tile([128, 128], bf16, name="ident_sb")
            nc.scalar.dma_start(out=ident_sb, in_=ident_d)
            dummy_sb = const.tile([1, 1], f32, name="dummy_sb")
            nc.vector.memset(dummy_sb, 0.0)
            # per-token rms scale, token-major columns (for v eviction)
            scols_sb = const.tile([128, JT], f32, name="scols_sb")
            nc.scalar.dma_start(out=scols_sb, in_=scols_d)
            # v-projection weights, resident: [128 hid-part, KH, HPC*128]
            wv_sb = const.tile([128, KH, HPC * 128], bf16, name="wv_sb")
            nc.sync.dma_start(out=wv_sb, in_=wv_d)

            # persistent K / V (token history)
            kT = kv.tile([128, HPC, T], bf16, name="kT")
            v_sb = kv.tile([128, HPC, JT, 129], bf16, name="v_sb")

            # ---- per-chunk helper emitters ------------------------------
            def emit_sbc(ci):
                tok0, tc = chunk_list[ci]
                s_bc = spool.tile([128, 512], f32, tag="sbc", bufs=2,
                                  name=f"sbc_{ci}")
                nc.scalar.dma_start(out=s_bc[:, 0:tc],
                                    in_=sbc_d[:, tok0:tok0 + tc])
                return s_bc

            def emit_xt_loads(ci):
                """hid-major x tiles, one fused DMA (sync queue); ages
                rows pre-patched by the host."""
                tok0, tc = chunk_list[ci]
                xall = xpool.tile([128, KH, 512], bf16, tag="xT",
                                  name=f"xT_{ci}")
                for k in range(KH):
                    nc.sync.dma_start(
                        out=xall[:, k, 0:tc],
                        in_=xT_d[k * 128:(k + 1) * 128, tok0:tok0 + tc])
                return xall

            def emit_cos_sin(ci):
                tok0, tc = chunk_list[ci]
                cos_sb = qkp.tile([128, 512], bf16, tag="cos", bufs=2,
                                  name=f"cos_{ci}")
                nc.scalar.dma_start(out=cos_sb[:, 0:tc],
                                    in_=cos_d[:, tok0:tok0 + tc])
                sin_sb = qkp.tile([128, 512], bf16, tag="sin", bufs=2,
                                  name=f"sin_{ci}")
                nc.scalar.dma_start(out=sin_sb[:, 0:tc],
                                    in_=sin_d[:, tok0:tok0 + tc])
                return cos_sb, sin_sb

            # ---- chunk 0 prologue ---------------------------------------
            s_bc = emit_sbc(0)
            xTt = emit_xt_loads(0)
            cos_sb, sin_sb = emit_cos_sin(0)

            rs_tiles = []
            for ci, (tok0, tc) in enumerate(chunk_list):
                last = ci == len(chunk_list) - 1
                NT = tc // 128

                # ---- W_out weight prefetch ------------------------------
                wots = []
                for oc in range(NO):
                    wot = wop.tile([128, NCOMB, 512], bf16, tag="wo",
                                   name=f"wo_{ci}_{oc}")
                    nc.scalar.dma_start(out=wot, in_=wo_d[oc])
                    wots.append(wot)

                # ---- v projection (token-major) -------------------------
                for tsub in range(NT):
                    pv = ps_mm.tile([128, HPC * 128], f32, tag="a",
                                    name=f"pv_{ci}_{tsub}")
                    for k in range(KH):
                        nc.tensor.matmul(
                            pv,
                            lhsT=xTt[:, k, tsub * 128:(tsub + 1) * 128],
                            rhs=wv_sb[:, k, :],
                            start=(k == 0), stop=(k == KH - 1))
                    j = tok0 // 128 + tsub
                    for h in range(HPC):
                        nc.vector.tensor_scalar_mul(
                            v_sb[:, h, j, 0:128], pv[:, h * 128:(h + 1) * 128],
                            scols_sb[:, j:j + 1])
                        nc.vector.memset(v_sb[:, h, j, 128:129], 1.0)

                # ---- fused W_in matmul (transposed out) -----------------
                # m order: g1_0, g2_0, ..., g1_7, g2_7, qA, qB, kA, kB
                silu_prev = None
                qk_raw = {}
                comb = [None] * NCOMB
                for m in range(2 * NFF + 2 * HPC):
                    wmt = wmp.tile([128, KH, 128], bf16, tag="wm",
                                   name=f"wm_{ci}_{m}")
                    nc.sync.dma_start(out=wmt, in_=win_d[m])
                    pm = ps_mm.tile([128, tc], f32, tag="a",
                                    name=f"pm_{ci}_{m}")
                    for k in range(KH):
                        nc.tensor.matmul(pm, lhsT=wmt[:, k, :],
                                         rhs=xTt[:, k, 0:tc],
                                         start=(k == 0),
                                         stop=(k == KH - 1))
                    if m < 2 * NFF and m % 2 == 0:      # g1
                        g1t = evictp.tile([128, 512], bf16, tag="g1",
                                          name=f"g1_{ci}_{m//2}")
                        nc.vector.tensor_mul(g1t[:, 0:tc], pm, s_bc[:, 0:tc])
                        st = evictp.tile([128, 512], bf16, tag="silu",
                                         name=f"silu_{ci}_{m//2}")
                        nc.scalar.activation(st[:, 0:tc], g1t[:, 0:tc],
                                             AF.Silu)
                        silu_prev = st
                    elif m < 2 * NFF:                    # g2
                        p = m // 2
                        g2t = evictp.tile([128, 512], bf16, tag="g2",
                                          name=f"g2_{ci}_{p}")
                        nc.vector.tensor_mul(g2t[:, 0:tc], pm, s_bc[:, 0:tc])
                        ct = combp.tile([128, 512], bf16, tag="comb",
                                        name=f"comb_{ci}_{p}")
                        nc.vector.tensor_mul(ct[:, 0:tc], silu_prev[:, 0:tc],
                                             g2t[:, 0:tc])
                        comb[p] = ct
                    else:                                # q or k
                        qi = m - 2 * NFF
                        qk_raw[qi] = qkp.tile([128, 512], bf16, tag="qkraw",
                                              name=f"qkraw_{ci}_{qi}")
                        nc.vector.tensor_mul(qk_raw[qi][:, 0:tc], pm,
                                             s_bc[:, 0:tc])

                # warm the Exp table off the critical path
                dwarm = spool.tile([1, 1], f32, tag="dwarm", bufs=2,
                                   name=f"dwarm_{ci}")
                nc.scalar.activation(dwarm, dummy_sb, AF.Exp, scale=SCALE)

                # ---- rope ----------------------------------------------
                qT = qkp.tile([128, HPC, 512], bf16, tag="qT", bufs=2,
                              name=f"qT_{ci}")
                # (qi, destination slice): q -> qT chunk, k -> resident kT
                rope_jobs = [(h, qT[:, h, 0:tc]) for h in range(HPC)]
                rope_jobs += [(HPC + h, kT[:, h, tok0:tok0 + tc])
                              for h in range(HPC)]
                for qi, dst in rope_jobs:
                    src = qk_raw[qi]
                    psw = ps_misc.tile([128, tc], f32, tag="a",
                                       name=f"psw_{ci}_{qi}")
                    nc.tensor.matmul(psw, lhsT=swap_sb, rhs=src[:, 0:tc],
                                     start=True, stop=True)
                    rt1 = qkp.tile([128, 512], bf16, tag="rt1", bufs=2,
                                   name=f"rt1_{ci}_{qi}")
                    nc.vector.tensor_mul(rt1[:, 0:tc], psw, sin_sb[:, 0:tc])
                    rt2 = qkp.tile([128, 512], bf16, tag="rt2", bufs=2,
                                   name=f"rt2_{ci}_{qi}")
                    nc.vector.tensor_mul(rt2[:, 0:tc], src[:, 0:tc],
                                         cos_sb[:, 0:tc])
                    nc.vector.tensor_add(dst, rt1[:, 0:tc], rt2[:, 0:tc])
                qk_raw = {}

                # ---- prefetch next chunk inputs (early issue slots) -----
                if not last:
                    s_bc_n = emit_sbc(ci + 1)
                    xTt_n = emit_xt_loads(ci + 1)
                    cs_n = emit_cos_sin(ci + 1)

                # ---- causal attention + output proj, per sub-chunk ------
                for (q0, tcs) in SUBS[ci]:
                    qrel = q0 - tok0
                    j0 = q0 // 128        # first diagonal k-block index
                    NB = tcs // 128
                    kmax = (q0 + tcs) // 128
                    seg = tcs // NCORES
                    for h in range(HPC):
                        pa = [ps_attn.tile([128, 129], f32, tag="attn",
                                           name=f"pa_{ci}_{q0}_{h}_{i}")
                              for i in range(NB)]

                        def emit_score(j):
                            psc = ps_misc.tile([128, tcs], f32, tag="a",
                                               name=f"psc_{ci}_{q0}_{h}_{j}")
                            nc.tensor.matmul(
                                psc, lhsT=kT[:, h, j * 128:(j + 1) * 128],
                                rhs=qT[:, h, qrel:qrel + tcs],
                                start=True, stop=True)
                            return psc

                        pscs = [emit_score(0)]
                        for j in range(kmax):
                            psc_cur = pscs[j]
                            if j + 1 < kmax:
                                pscs.append(emit_score(j + 1))
                            pT = ppool.tile([128, 512], bf16, tag="p",
                                            name=f"pT_{ci}_{q0}_{h}_{j}")
                            nc.scalar.activation(pT[:, 0:tcs], psc_cur,
                                                 AF.Exp, scale=SCALE)
                            D = j * 128 - q0
                            if D >= 0:
                                # triangle mask on the diagonal block only
                                nc.vector.tensor_mul(
                                    pT[:, D:D + 128], pT[:, D:D + 128],
                                    tri_sb)
                            for b in range(NB):
                                jmax_b = j0 + b
                                if j > jmax_b:
                                    continue  # fully-masked block: skip
                                nc.tensor.matmul(
                                    pa[b],
                                    lhsT=pT[:, b * 128:(b + 1) * 128],
                                    rhs=v_sb[:, h, j, :],
                                    start=(j == 0), stop=(j == jmax_b))
                        # normalize + transpose into comb tiles
                        for b in range(NB):
                            li = attnp.tile([128, 1], f32, tag="l",
                                            name=f"l_{ci}_{q0}_{h}_{b}")
                            nc.vector.reciprocal(li, pa[b][:, 128:129])
                            at = attnp.tile([128, 128], bf16, tag="at",
                                            name=f"at_{ci}_{q0}_{h}_{b}")
                            nc.vector.tensor_scalar_mul(
                                at, pa[b][:, 0:128], li)
                            ptr = ps_misc.tile([128, 128], bf16, tag="a",
                                               name=f"ptr_{ci}_{q0}_{h}_{b}")
                            nc.tensor.transpose(ptr, at, ident_sb)
                            if comb[NFF + h] is None:
                                comb[NFF + h] = combp.tile(
                                    [128, 512], bf16, tag="comb",
                                    name=f"comb_at_{ci}_{h}")
                            col0 = qrel + b * 128
                            nc.vector.tensor_copy(
                                comb[NFF + h][:, col0:col0 + 128], ptr)

                    # ---- output projection for this sub-chunk -----------
                    NTS = tcs // 128
                    whole = tcs == 512
                    if whole:
                        acc_oc = [dram.tile([tcs, 512], bf16, tag="acc",
                                            bufs=8,
                                            name=f"acc_{ci}_{q0}_{oc}")
                                  for oc in range(NO)]
                    else:
                        acc_one = dram.tile([tcs, HID], bf16,
                                            tag=f"acc1_{tcs}", bufs=2,
                                            name=f"acc1_{ci}_{q0}")
                    for oc in range(NO):
                        wot = wots[oc]
                        for tsub in range(NTS):
                            po = ps_out.tile([128, 512], f32, tag="out",
                                             name=f"po_{ci}_{q0}_{oc}_{tsub}")
                            csl = slice(qrel + tsub * 128,
                                        qrel + (tsub + 1) * 128)
                            for kc in range(NCOMB):
                                nc.tensor.matmul(
                                    po, lhsT=comb[kc][:, csl],
                                    rhs=wot[:, kc, :],
                                    start=(kc == 0), stop=(kc == NCOMB - 1))
                            ost = outp.tile([128, 512], bf16, tag="ost",
                                            name=f"ost_{ci}_{q0}_{oc}_{tsub}")
                            nc.vector.tensor_copy(ost, po)
                            r0 = tsub * 128
                            if whole:
                                nc.scalar.dma_start(
                                    out=acc_oc[oc][r0:r0 + 128, :], in_=ost)
                            else:
                                nc.scalar.dma_start(
                                    out=acc_one[r0:r0 + 128,
                                                oc * 512:(oc + 1) * 512],
                                    in_=ost)

                    # ---- reduce-scatter this sub-chunk ------------------
                    if whole:
                        for oc in range(NO):
                            rs_c = dram.tile([seg, 512], bf16, tag="rs",
                                             bufs=32,
                                             name=f"rs_{ci}_{q0}_{oc}")
                            nc.gpsimd.collective_compute(
                                "ReduceScatter",
                                mybir.AluOpType.add,
                                replica_groups=[list(range(NCORES))],
                                ins=[acc_oc[oc][:, :]],
                                outs=[rs_c[:, :]],
                            )
                            rs_tiles.append((q0, tcs, oc, rs_c))
                    else:
                        rs_c = dram.tile([seg, HID], bf16,
                                         tag=f"rs1_{tcs}", bufs=2,
                                         name=f"rs1_{ci}_{q0}")
                        nc.gpsimd.collective_compute(
                            "ReduceScatter",
                            mybir.AluOpType.add,
                            replica_groups=[list(range(NCORES))],
                            ins=[acc_one[:, :]],
                            outs=[rs_c[:, :]],
                        )
                        rs_tiles.append((q0, tcs, None, rs_c))

                if not last:
                    s_bc = s_bc_n
                    xTt = xTt_n
                    cos_sb, sin_sb = cs_n

            # deferred final output DMAs; gpsimd SWDGE path so the HW DGE
            # queue counters never chain later DMAs behind the collectives
            for (q0, tcs, oc, rs_c) in rs_tiles:
                r0 = q0 // NCORES
                if oc is not None:
                    nc.gpsimd.dma_start(
                        out=out_d[oc, r0:r0 + tcs // NCORES, :],
                        in_=rs_c[:, :])
                else:
                    for o2 in range(NO):
                        nc.gpsimd.dma_start(
                            out=out_d[o2, r0:r0 + tcs // NCORES, :],
                            in_=rs_c[:, o2 * 512:(o2 + 1) * 512])

    nc.compile()
    return nc


def _prep_in_maps(x, normed_ages, sin, cos, norm_w, W_in, W_out):
    """Shard + preprocess inputs into per-core in_maps (numpy only)."""
    T = x.shape[0]
    x = np.asarray(x, np.float32)
    # host-side RMSNorm scale, fp32 exactly like the reference
    s = 1.0 / np.sqrt(np.mean(x * x, axis=1) + EPS)          # [T]
    sbc = np.ascontiguousarray(
        np.broadcast_to(s[None, :], (128, T))).astype(np.float32)
    scols = np.ascontiguousarray(
        s.reshape(T // 128, 128).T).astype(np.float32)       # [128, JT]

    xT_bf = np.ascontiguousarray(x.T).astype(BF16)
    # ages overwrite: patch the last two hid rows with a12 / s so the
    # eviction-side multiply by s restores a12 exactly
    a1 = np.asarray(normed_ages, np.float32)
    xT_bf[HID - 2, :] = (a1 / s).astype(BF16)
    xT_bf[HID - 1, :] = (a1 * a1 / s).astype(BF16)

    cos_t = np.ascontiguousarray(cos.reshape(T, HD).T).astype(BF16)
    sin_t = np.ascontiguousarray(sin.reshape(T, HD).T).astype(BF16)

    sw = np.zeros((128, 128), np.float32)
    idx = np.arange(0, 128, 2)
    sw[idx + 1, idx] = -1.0   # lhsT[2i+1, 2i] = -1
    sw[idx, idx + 1] = 1.0    # lhsT[2i, 2i+1] = +1
    swapmat = sw.astype(BF16)

    maskbase = (np.arange(896)[None, :] - 384 >=
                np.arange(128)[:, None]).astype(BF16)
    identity = np.eye(128, dtype=np.float32).astype(BF16)

    # norm_w folded into W_in except the last two hid columns (the
    # normed_ages overwrite bypasses the norm weight).
    def fold(wrows):
        w = wrows * norm_w[None, :]
        w[:, HID - 2:] = wrows[:, HID - 2:]
        return w

    q_base = 2 * INTER
    k_base = 2 * INTER + HID
    v_base = 2 * INTER + 2 * HID

    in_maps = []
    for core in range(NCORES):
        f0 = FPC * core
        h0 = HPC * core
        rows = []
        for p in range(NFF):
            rows.append(W_in[f0 + p * 128: f0 + (p + 1) * 128])           # g1_p
            rows.append(W_in[INTER + f0 + p * 128:
                             INTER + f0 + (p + 1) * 128])                 # g2_p
        for h in range(HPC):
            rows.append(W_in[q_base + (h0 + h) * HD:
                             q_base + (h0 + h + 1) * HD])                 # q
        for h in range(HPC):
            rows.append(W_in[k_base + (h0 + h) * HD:
                             k_base + (h0 + h + 1) * HD])                 # k
        w_used = fold(np.concatenate(rows, axis=0))                       # [2560, HID]
        nm = 2 * NFF + 2 * HPC
        # [m, p(hid-in-tile), k, j(row-in-tile)] so each partition is linear
        w_in_t = np.ascontiguousarray(
            w_used.reshape(nm, 128, KH, 128).transpose(0, 3, 2, 1)
        ).astype(BF16)

        wv = fold(W_in[v_base + h0 * HD: v_base + (h0 + HPC) * HD])       # [256, HID]
        w_v_t = np.ascontiguousarray(
            wv.reshape(HPC * 128, KH, 128).transpose(2, 1, 0)).astype(BF16)

        # W_out columns in comb order: ff block, then attn heads
        cols = list(range(HID + f0, HID + f0 + FPC))
        for h in range(HPC):
            cols += list(range((h0 + h) * HD, (h0 + h + 1) * HD))
        w_o_loc_t = np.ascontiguousarray(W_out[:, cols].T)                # [1280, HID]
        # [oc, p(c-in-tile), kc, ow] so each partition is linear per oc
        w_out_t = np.ascontiguousarray(
            w_o_loc_t.reshape(NCOMB, 128, HID // 512, 512)
            .transpose(2, 1, 0, 3)).astype(BF16)

        in_maps.append({
            "xt": xT_bf, "sbc": sbc, "scols": scols,
            "w_in_t": w_in_t, "w_v_t": w_v_t, "w_out_t": w_out_t,
            "cos_t": cos_t, "sin_t": sin_t,
            "swapmat": swapmat, "maskbase": maskbase, "identity": identity,
        })
    return in_maps


_NC_CACHE = {}


def get_nc(T=T_FULL):
    if T not in _NC_CACHE:
        _NC_CACHE[T] = _build_nc(T)
    return _NC_CACHE[T]


def run(x, normed_ages, sin, cos, norm_w, W_in, W_out, T=T_FULL,
        trace=False):
    from concourse.bass_utils import run_bass_kernel_spmd
    nc = get_nc(T)
    in_maps = _prep_in_maps(x, normed_ages, sin, cos, norm_w, W_in, W_out)
    res = run_bass_kernel_spmd(nc, in_maps, list(range(NCORES)), trace=trace)
    # results[i]["out"][oc, tok0//8 + t] holds reduced rows
    # [tok0 + i*seg + t, oc*512:(oc+1)*512] for each chunk
    out = np.empty((T, HID), np.float32)
    for i in range(NCORES):
        oi = np.asarray(res.results[i]["out"], np.float32)
        for subs in SUBS:
            for (q0, tcs) in subs:
                seg = tcs // NCORES
                r0 = q0 // NCORES
                for oc in range(HID // 512):
                    out[q0 + i * seg: q0 + (i + 1) * seg,
                        oc * 512:(oc + 1) * 512] = oi[oc, r0:r0 + seg]
    return out, res


def kernel(x, normed_ages, sin, cos, norm_w, W_in, W_out):
    out, _ = run(x, normed_ages, sin, cos, norm_w, W_in, W_out)
    return out


if __name__ == "__main__":
    import reference
    inputs = reference.setup_inputs()
    inputs = {k: np.asarray(v) for k, v in inputs.items()}
    expected = np.asarray(reference.reference(**inputs))
    got = kernel(**inputs)
    rel = np.linalg.norm(got - expected) / np.linalg.norm(expected)
    print("rel", rel)
